# revision 1
# baseline (speedup 1.0000x reference)
"""Distributed causal self-attention with RoPE for 8 TRN2 NeuronCores.

Sharding (Megatron-style, per the hint): head-parallel. Core c owns heads
(2c, 2c+1) for both batch elements. c_attn is column-parallel (each core
computes q/k/v only for its heads from the full x), attention is fully local
per head, and c_proj is row-parallel (each core multiplies its 128 head
channels into a full-width partial output). The 8 partial outputs are summed
on the host during unsharding — no on-device collective is needed, which
beats a 16.8MB AllReduce (~190us) by a wide margin.

Per-core kernel layout choices:
  - x is passed pre-transposed as xT [C, B*T] (bf16): QKV runs as
    qT = Wq^T @ xT giving q^T in [head_dim, t] layout, which is exactly the
    lhsT/rhs layout the attention matmuls want (contraction over d).
  - v is computed in [t, d] layout (lhsT = xT tile), augmented with a
    ones-column so the PV matmul yT = v_aug^T @ exp(S^T) yields the softmax
    denominator in its last row for free.
  - RoPE is applied in [d, t] layout: the half-rotation is a partition swap
    done with two SBUF->SBUF DMAs, then 3 elementwise ops against
    host-precomputed cos/sin tables.
  - Softmax skips the running-max subtraction: scores are ~N(0,1) after the
    1/sqrt(d) scale, so exp never overflows fp32; exp runs on the scalar
    engine straight out of PSUM, writing bf16.
  - Causality is exploited at tile granularity (strictly-upper tiles are
    skipped; diagonal tiles stream partial columns and get a triangular
    mask multiply after exp).
  - The softmax denominators are inverted as exp(-ln(den)) on the scalar
    engine and partition-broadcast with two accumulating K=1 matmuls.
  - c_proj runs transposed (out^T = Wo^T @ yT); its bias (plus the folded
    v-bias) is added once on the host after the partial sums.
  - Emission order is software-pipelined for Tile's static engine queues:
    PV lags S by several iterations, each block's normalize/proj matmuls
    are deferred into the next block's stream, and the next groups' QKV
    matmuls are spliced in as feeders so the tensor engine never idles
    (keeping the PE HAM clock warm).
"""

import os
import sys
import types

import numpy as np
import ml_dtypes

import concourse.bass as bass
import concourse.mybir as mybir
from concourse.tile import TileContext
from concourse.vector_clock import ScopedClock

BF16 = mybir.dt.bfloat16
F32 = mybir.dt.float32

N_CORES = 8
B, T, C = 2, 2048, 1024
H, D = 16, 64
HPC = H // N_CORES  # heads per core
HD = HPC * D  # local head width = 128
TT = B * T  # flattened tokens = 4096
NK = C // 128  # contraction tiles for QKV
NBLK = T // 512  # tq blocks per batch
NTK = T // 128  # tk tiles per batch
SCALE = float(D) ** -0.5
ROPE_THETA = 10000.0


def _install_axon_hooks_shim():
    """Best-effort: some environments lack antenv.axon_hooks, which
    run_bass_kernel_spmd imports when BASS_TRACE is set. Provide a minimal
    implementation backed by the slim trn boot module if available."""
    try:
        import antenv.axon_hooks  # noqa: F401

        return
    except ImportError:
        pass
    try:
        hook = [None]
        mod = types.ModuleType("antenv.axon_hooks")
        mod.set_axon_ntff_profile_hook = lambda h: hook.__setitem__(0, h)
        mod.get_axon_ntff_profile_hook = lambda: hook[0]
        try:
            from trn_agent_boot.trn_boot import _ntff_profile_via_ctypes

            so = "/opt/axon/libaxon_pjrt.so"
            if os.path.exists(so):
                hook[0] = _ntff_profile_via_ctypes(so)
        except Exception:
            pass
        sys.modules["antenv.axon_hooks"] = mod
        import antenv

        antenv.axon_hooks = mod
    except Exception:
        pass


_install_axon_hooks_shim()


class _TileContextSplitDrain(TileContext):
    """This walrus build rejects >2 sync-waits on one instruction; the Tile
    kernel-tail drain can carry more. Split them across single-wait NOPs."""

    def _drain_and_barrier(self, tick_clock, wait_clock):
        drain_inst = self.nc.sync.drain()
        wait_clock.add_sem_waits(
            drain_inst.ins, ScopedClock({None: tick_clock.global_clock})
        )
        waits = list(drain_inst.ins.sync_info.on_wait)
        if len(waits) > 1:
            drain_inst.ins.sync_info.on_wait[:] = waits[:1]
            for w in waits[1:]:
                nop = self.nc.sync.nop(nofuse=True)
                nop.ins.sync_info = mybir.SyncInfo(on_wait=[w], on_update=[])

        self.nc.all_engine_barrier()
        assert self.sems is not None
        popped = self.nc._tile_sem_poison_stack.pop()
        assert popped is self._sem_poison
        self.nc.clear_and_free_semaphores(list(self.sems.allocated().values()))
        self.nc.all_engine_barrier()


def _split_excess_waits(nc: bass.Bass, limit: int = 1) -> int:
    """This walrus build encodes only a small number of sync-waits per
    instruction; Tile's semaphore assignment can attach more. Hoist excess
    waits onto same-engine NOPs placed immediately before the instruction —
    semantically identical since engine queues execute in order."""
    import bass_rust

    ctr = 0
    for fn in nc.m.functions:
        for bb in fn.blocks:
            insts = bb.instructions
            new = []
            for inst in insts:
                si = inst.sync_info
                waits = list(si.on_wait) if si is not None else []
                if len(waits) > limit:
                    keep = waits[-limit:]
                    extra = waits[: -limit]
                    for s in range(0, len(extra), limit):
                        chunk = extra[s : s + limit]
                        ctr += 1
                        nop = bass_rust.InstNoOp(
                            name=f"I-wsplit{ctr}",
                            engine=inst.engine,
                            ins=[],
                            outs=[],
                            sync_info=mybir.SyncInfo(
                                on_wait=chunk, on_update=[]
                            ),
                        )
                        nc.register_instruction(nop)
                        new.append(nop)
                    si.on_wait[:] = keep
                new.append(inst)
            insts[:] = new
    return ctr


def _build_nc() -> bass.Bass:
    nc = bass.Bass()

    xT = nc.declare_dram_parameter("xT", [C, TT], BF16, isOutput=False)
    wq = nc.declare_dram_parameter("wq", [128, C], BF16, isOutput=False)
    wk = nc.declare_dram_parameter("wk", [128, C], BF16, isOutput=False)
    wv = nc.declare_dram_parameter("wv", [128, C], BF16, isOutput=False)
    wo = nc.declare_dram_parameter("wo", [HD, C], BF16, isOutput=False)
    bq = nc.declare_dram_parameter("bq", [128, 1], F32, isOutput=False)
    bk = nc.declare_dram_parameter("bk", [128, 1], F32, isOutput=False)
    cosd = nc.declare_dram_parameter("cosT", [128, TT], BF16, isOutput=False)
    sind = nc.declare_dram_parameter("sinT", [128, TT], BF16, isOutput=False)
    trid = nc.declare_dram_parameter("tri", [128, 128], BF16, isOutput=False)
    eyed = nc.declare_dram_parameter("eye", [128, 128], BF16, isOutput=False)
    outd = nc.declare_dram_parameter("out", [C, TT], BF16, isOutput=True)

    Exp = mybir.ActivationFunctionType.Exp
    Copy = mybir.ActivationFunctionType.Copy
    Ident = mybir.ActivationFunctionType.Identity
    Ln = mybir.ActivationFunctionType.Ln

    with _TileContextSplitDrain(nc) as tc:
        with (
            tc.tile_pool(name="consts", bufs=1) as cp,
            tc.tile_pool(name="xt", bufs=NK) as xtp,
            tc.tile_pool(name="qk", bufs=1) as qkp,
            tc.tile_pool(name="qsw", bufs=4) as qswp,
            tc.tile_pool(name="rot", bufs=1) as rotp,
            tc.tile_pool(name="vaug", bufs=B * NBLK) as vaugp,
            tc.tile_pool(name="apool", bufs=6) as apool,
            tc.tile_pool(name="yb", bufs=1) as ybp,
            tc.tile_pool(name="rsmall", bufs=2) as rsp,
            tc.tile_pool(name="rbig", bufs=2) as rbp,
            tc.tile_pool(name="osb", bufs=6) as osbp,
            tc.tile_pool(name="psmix", bufs=2, space="PSUM") as psmix,
            tc.tile_pool(name="pss", bufs=2, space="PSUM") as pss,
            tc.tile_pool(name="psy0", bufs=1, space="PSUM") as psy0,
            tc.tile_pool(name="psy1", bufs=1, space="PSUM") as psy1,
        ):
            # ---- constants / weights -------------------------------------
            wq_t = cp.tile([128, C], BF16, tag="wq")
            wk_t = cp.tile([128, C], BF16, tag="wk")
            wv_t = cp.tile([128, C], BF16, tag="wv")
            wo_t = cp.tile([HD, C], BF16, tag="wo")
            bq_t = cp.tile([128, 1], F32, tag="bq")
            bk_t = cp.tile([128, 1], F32, tag="bk")
            cos_t = cp.tile([128, TT], BF16, tag="cos")
            sin_t = cp.tile([128, TT], BF16, tag="sin")
            tri_t = cp.tile([128, 128], BF16, tag="tri")
            eye_t = cp.tile([128, 128], BF16, tag="eye")
            e2a_t = cp.tile([1, 128], BF16, tag="e2a")
            e2b_t = cp.tile([1, 128], BF16, tag="e2b")

            # Input DMAs: a single HWDGE queue streams ~1MB in ~30us, so a
            # big table DMA clogs its queue and everything ordered behind it
            # waits. Chunk to <=256KB, round-robin the three issuing engines,
            # and order critical-first (wq/wk + xt piece A unblock the first
            # QKV chunk; cos/sin are only needed ~10us later).
            idma_engs = (nc.sync, nc.scalar, nc.gpsimd)
            idma_i = [0]

            def idma(dst_ap, src_ap):
                idma_engs[idma_i[0] % 3].dma_start(out=dst_ap, in_=src_ap)
                idma_i[0] += 1

            xts = []
            for _ in range(NK):
                xt_tile = xtp.tile([128, TT], BF16, tag="xt")
                xts.append(xt_tile)

            idma(eye_t[:, :], eyed[:, :])
            idma(wq_t[:, :], wq[:, :])
            idma(wk_t[:, :], wk[:, :])
            for k in range(NK):
                idma(xts[k][:, 0:512], xT[k * 128 : (k + 1) * 128, 0:512])
            for dst, srcd in ((bq_t, bq), (bk_t, bk), (tri_t, trid)):
                idma(dst[:, :], srcd[:, :])
            idma(wv_t[:, :], wv[:, :])
            idma(cos_t[:, 0:1024], cosd[:, 0:1024])
            idma(sin_t[:, 0:1024], sind[:, 0:1024])
            bounds = [512, 1536, 2560, 3584, TT]
            for k in range(NK):
                idma(xts[k][:, 512:1536], xT[k * 128 : (k + 1) * 128, 512:1536])
            for c in range(1, 4):
                ccols = slice(c * 1024, (c + 1) * 1024)
                idma(cos_t[:, ccols], cosd[:, ccols])
                idma(sin_t[:, ccols], sind[:, ccols])
            for p in range(1, 4):
                pcols = slice(bounds[p], bounds[p + 1])
                for k in range(NK):
                    idma(xts[k][:, pcols], xT[k * 128 : (k + 1) * 128, pcols])
            idma(wo_t[:, :], wo[:, :])

            # e2a/e2b: indicator rows of each head's 64-partition block; two
            # accumulating K=1 matmuls broadcast each head's 1/denom row onto
            # its partition block of one [128, 512] PSUM tile.
            nc.vector.memset(e2a_t[:, :], 0.0)
            nc.vector.memset(e2b_t[:, :], 0.0)
            nc.vector.memset(e2a_t[0:1, 0:64], 1.0)
            nc.vector.memset(e2b_t[0:1, 64:128], 1.0)

            q_sb = qkp.tile([128, TT], BF16, tag="q_sb")
            k_sb = qkp.tile([128, TT], BF16, tag="k_sb")
            vt_sb = qkp.tile([128, TT], BF16, tag="vt_sb")
            qr = rotp.tile([128, TT], BF16, tag="qr")
            kr = rotp.tile([128, TT], BF16, tag="kr")
            yb = ybp.tile([HD, TT], BF16, tag="yb")
            vaugs = [None] * (B * NBLK)

            def qkv_one(g, dst, w_t, b_t):
                cols = slice(g * 512, (g + 1) * 512)
                ps = psmix.tile([128, 512], F32, tag="mix")
                for k in range(NK):
                    nc.tensor.matmul(
                        ps[:, :],
                        w_t[:, k * 128 : (k + 1) * 128],
                        xts[k][:, cols],
                        start=(k == 0),
                        stop=(k == NK - 1),
                    )
                if b_t is None:
                    nc.vector.tensor_copy(dst[:, cols], ps[:, :])
                else:
                    nc.vector.tensor_scalar_add(dst[:, cols], ps[:, :], b_t[:, 0:1])

            def qkv_chunk(g):
                qkv_one(g, q_sb, wq_t, bq_t)
                qkv_one(g, k_sb, wk_t, bk_t)
                qkv_one(g, vt_sb, wv_t, None)

            def rope_chunk(g):
                cols = slice(g * 512, (g + 1) * 512)
                for src_t, dst_t in ((q_sb, qr), (k_sb, kr)):
                    sw = qswp.tile([128, 512], BF16, tag="sw")
                    for hb in range(HPC):
                        p = hb * 64
                        nc.gpsimd.dma_start(
                            out=sw[p : p + 32, :],
                            in_=src_t[p + 32 : p + 64, cols],
                        )
                        nc.gpsimd.dma_start(
                            out=sw[p + 32 : p + 64, :],
                            in_=src_t[p : p + 32, cols],
                        )

                    nc.vector.tensor_mul(
                        dst_t[:, cols], src_t[:, cols], cos_t[:, cols]
                    )
                    nc.vector.tensor_mul(sw[:, :], sw[:, :], sin_t[:, cols])
                    nc.vector.tensor_add(dst_t[:, cols], dst_t[:, cols], sw[:, :])

            def v_tiles4(g):
                # transpose v^T[:, g*512:(g+1)*512] into one [t,d] vaug group
                # tile (4 tk-tiles x [64+1 | 64+1] layout) via PE transposes
                # batched through one PSUM slot, one merged copy, one memset.
                ps = psmix.tile([128, 512], BF16, tag="mix")
                for j in range(4):
                    tt = 4 * g + j
                    nc.tensor.transpose(
                        ps[:, j * 128 : (j + 1) * 128],
                        vt_sb[:, tt * 128 : (tt + 1) * 128],
                        eye_t[:, :],
                    )
                vg = vaugp.tile([128, 4 * 130], BF16, tag="vaug")
                v4 = vg[:, :].rearrange("p (t b c) -> p t b c", t=4, b=2)
                nc.vector.memset(v4[:, :, :, 64:65], 1.0)
                nc.vector.tensor_copy(
                    v4[:, :, :, 0:64],
                    ps[:, :].rearrange("p (t b c) -> p t b c", t=4, b=2),
                )
                vaugs[g] = vg

            def attn_block(b, blk, pending_pe, feeders=()):
                # Software-pipelined emission: Tile engine queues run in
                # static program order, so PV(j) directly after S(j) would
                # stall the PE on exp(j) every iteration. Emit PV lagging S
                # by 2 iterations, and splice the previous block's R/proj
                # matmuls (pending_pe) into this block's stream so their
                # ACT/DVE dependencies are long satisfied when the PE
                # reaches them.
                LAG = 5
                gb = b * T
                base = gb + blk * 512
                ktiles = 4 * (blk + 1)
                feeders = list(feeders)
                feed_at = {2, max(3, ktiles // 2), ktiles - 1}
                yt0 = psy0.tile([65, 512], F32, tag="yt0")
                yt1 = psy1.tile([65, 512], F32, tag="yt1")
                stage = []  # (tk, c0, A)

                def emit_pv(tk, c0, A):
                    vg = vaugs[b * NBLK + tk // 4]
                    vo = (tk % 4) * 130
                    nc.tensor.matmul(
                        yt0[0:65, c0:512], vg[:, vo : vo + 65], A[:, c0:512],
                        start=(tk == 0), stop=(tk == ktiles - 1),
                    )
                    nc.tensor.matmul(
                        yt1[0:65, c0:512], vg[:, vo + 65 : vo + 130],
                        A[:, 512 + c0 : 1024],
                        start=(tk == 0), stop=(tk == ktiles - 1),
                    )

                for tk in range(ktiles):
                    diag = tk >= blk * 4
                    c0 = (tk - blk * 4) * 128 if diag else 0
                    S = pss.tile([128, 1024], F32, tag="spair")
                    A = apool.tile([128, 1024], BF16, tag="apair")
                    kcol = slice(gb + tk * 128, gb + (tk + 1) * 128)
                    qcol = slice(base + c0, base + 512)
                    nc.tensor.matmul(
                        S[:, c0:512], kr[0:64, kcol], qr[0:64, qcol],
                        start=True, stop=True,
                    )
                    nc.tensor.matmul(
                        S[:, 512 + c0 : 1024], kr[64:128, kcol],
                        qr[64:128, qcol], start=True, stop=True,
                    )
                    if diag:
                        s3 = S[:, :].rearrange("p (h c) -> p h c", h=2)[
                            :, :, c0:512
                        ]
                        a3 = A[:, :].rearrange("p (h c) -> p h c", h=2)[
                            :, :, c0:512
                        ]
                        nc.scalar.activation(a3, s3, Exp, scale=SCALE)
                        nc.vector.tensor_mul(
                            A[:, c0 : c0 + 128],
                            A[:, c0 : c0 + 128],
                            tri_t[:, :],
                        )
                        nc.vector.tensor_mul(
                            A[:, 512 + c0 : 512 + c0 + 128],
                            A[:, 512 + c0 : 512 + c0 + 128],
                            tri_t[:, :],
                        )
                    else:
                        nc.scalar.activation(A[:, :], S[:, :], Exp, scale=SCALE)
                    stage.append((tk, c0, A))
                    if len(stage) > LAG:
                        emit_pv(*stage.pop(0))
                    if pending_pe and tk in (1, max(3, ktiles // 2 + 1)):
                        pending_pe.pop(0)()
                    if feeders and tk in feed_at:
                        feeders.pop(0)()
                while feeders:
                    feeders.pop(0)()
                while stage:
                    emit_pv(*stage.pop(0))
                while pending_pe:
                    pending_pe.pop(0)()

                # Finalize. Free the yT PSUM banks as early as possible
                # (next block's PV accumulation waits on them): copy the
                # unnormalized numerators out and take Ln of the denominators,
                # then run the normalization chain entirely in SBUF.
                # 1/den is exp(-ln(den)) on the scalar engine: a [1, N] op is
                # single-lane on every engine, and ACT streams it at 1
                # elem/cyc while DVE's reciprocal needs ~7 cyc/elem.
                nb = rbp.tile([128, 512], BF16, tag="nb")
                rf0 = rsp.tile([1, 512], F32, tag="rf0")
                rf1 = rsp.tile([1, 512], F32, tag="rf1")
                nc.scalar.activation(rf0[:, :], yt0[64:65, 0:512], Ln)
                nc.vector.tensor_copy(nb[0:64, :], yt0[0:64, 0:512])
                nc.scalar.activation(rf1[:, :], yt1[64:65, 0:512], Ln)
                nc.vector.tensor_copy(nb[64:128, :], yt1[0:64, 0:512])
                rb0 = rsp.tile([1, 512], BF16, tag="rb0")
                rb1 = rsp.tile([1, 512], BF16, tag="rb1")
                nc.scalar.activation(rb0[:, :], rf0[:, :], Exp, scale=-1.0)
                nc.scalar.activation(rb1[:, :], rf1[:, :], Exp, scale=-1.0)

                def pe_tail1():
                    Rp = psmix.tile([128, 512], F32, tag="mix")
                    nc.tensor.matmul(
                        Rp[:, :], e2a_t[0:1, :], rb0[0:1, :],
                        start=True, stop=False,
                    )
                    nc.tensor.matmul(
                        Rp[:, :], e2b_t[0:1, :], rb1[0:1, :],
                        start=False, stop=True,
                    )
                    nc.vector.tensor_mul(
                        yb[0:64, base : base + 512], nb[0:64, :], Rp[0:64, :]
                    )
                    nc.vector.tensor_mul(
                        yb[64:128, base : base + 512], nb[64:128, :],
                        Rp[64:128, :],
                    )
                    proj_block(b, blk, range(0, 4))

                def pe_tail2():
                    proj_block(b, blk, range(4, C // 128))

                return [pe_tail1, pe_tail2]

            def proj_block(b, blk, ccs):
                base = b * T + blk * 512
                for cc in ccs:
                    op = psmix.tile([128, 512], F32, tag="mix")
                    nc.tensor.matmul(
                        op[:, :],
                        wo_t[:, cc * 128 : (cc + 1) * 128],
                        yb[:, base : base + 512],
                        start=True, stop=True,
                    )
                    ob = osbp.tile([128, 512], BF16, tag="ob")
                    nc.vector.tensor_copy(ob[:, :], op[:, :])
                    nc.sync.dma_start(
                        out=outd[cc * 128 : (cc + 1) * 128, base : base + 512],
                        in_=ob[:, :],
                    )

            # Interleave: group 0's QKV runs as a prologue; every later
            # group's QKV pieces are spliced INTO the previous block's
            # attention stream (feeders) so the tensor engine always has
            # independent work between the exp-dependent PV matmuls — this
            # keeps the PE HAM clock warm. Each block's normalization and
            # projection matmuls are deferred into the next block's stream
            # (pending_pe).
            for g01 in (0, 1):
                qkv_chunk(g01)
                rope_chunk(g01)
                v_tiles4(g01)
            pending = []
            for g in range(B * NBLK):
                h = g + 2
                if h < B * NBLK:
                    feeders = [
                        lambda h=h: qkv_one(h, q_sb, wq_t, bq_t),
                        lambda h=h: (
                            qkv_one(h, k_sb, wk_t, bk_t), rope_chunk(h)
                        ),
                        lambda h=h: (
                            qkv_one(h, vt_sb, wv_t, None), v_tiles4(h)
                        ),
                    ]
                else:
                    feeders = []
                pending = attn_block(g // NBLK, g % NBLK, pending, feeders)
            for p in pending:
                p()

    _split_excess_waits(nc, limit=1)
    return nc


_NC_CACHE = None


def _get_nc() -> bass.Bass:
    global _NC_CACHE
    if _NC_CACHE is None:
        _NC_CACHE = _build_nc()
    return _NC_CACHE


def _prep_in_maps(x, w_attn, b_attn, w_proj, b_proj):
    bf = ml_dtypes.bfloat16
    x = np.asarray(x, np.float32)
    w_attn = np.asarray(w_attn, np.float32)
    b_attn = np.asarray(b_attn, np.float32)
    w_proj = np.asarray(w_proj, np.float32)
    b_proj = np.asarray(b_proj, np.float32)

    xT = np.ascontiguousarray(x.reshape(TT, C).T).astype(bf)

    freqs = 1.0 / ROPE_THETA ** (np.arange(0, D, 2, dtype=np.float64) / D)
    ang = np.arange(T, dtype=np.float64)[:, None] * freqs[None, :]  # [T, 32]
    cosb = np.cos(ang).T  # [32, T]
    sinb = np.sin(ang).T
    cos64 = np.concatenate([cosb, cosb], axis=0)  # rows 0:32 and 32:64
    sin64 = np.concatenate([-sinb, sinb], axis=0)  # signed for the rotation
    cos128 = np.concatenate([cos64, cos64], axis=0)  # two heads
    sin128 = np.concatenate([sin64, sin64], axis=0)
    cosT = np.ascontiguousarray(np.tile(cos128, (1, B))).astype(bf)
    sinT = np.ascontiguousarray(np.tile(sin128, (1, B))).astype(bf)

    r = np.arange(128)
    tri = (r[:, None] <= r[None, :]).astype(np.float32).astype(bf)
    eye = np.eye(128, dtype=np.float32).astype(bf)


    def karr(w):  # [C, 128] -> [128, C] with [p, k*128+j] = w[k*128+p, j]
        return np.ascontiguousarray(
            w.reshape(NK, 128, 128).transpose(1, 0, 2).reshape(128, C)
        ).astype(bf)

    maps = []
    for c in range(N_CORES):
        sl = slice(c * HD, (c + 1) * HD)
        maps.append(
            dict(
                xT=xT,
                wq=karr(w_attn[:, 0 * C : 1 * C][:, sl]),
                wk=karr(w_attn[:, 1 * C : 2 * C][:, sl]),
                wv=karr(w_attn[:, 2 * C : 3 * C][:, sl]),
                wo=np.ascontiguousarray(w_proj[sl, :]).astype(bf),
                bq=np.ascontiguousarray(
                    b_attn[0 * C : 1 * C][sl].reshape(128, 1)
                ).astype(np.float32),
                bk=np.ascontiguousarray(
                    b_attn[1 * C : 2 * C][sl].reshape(128, 1)
                ).astype(np.float32),
                cosT=cosT,
                sinT=sinT,
                tri=tri,
                eye=eye,
            )
        )
    return maps


def _gather(results, b_eff) -> np.ndarray:
    outT = np.sum(
        np.stack([np.asarray(r["out"], np.float32) for r in results]),
        axis=0,
        dtype=np.float64,
    )
    y = outT.reshape(C, B, T).transpose(1, 2, 0) + b_eff[None, None, :]
    return np.ascontiguousarray(y).astype(np.float32)


def _bias_eff(b_attn, w_proj, b_proj):
    # v's bias is dropped on-device: softmax rows sum to 1, so its effect on
    # the output is the constant b_v @ w_proj — fold it, with c_proj's own
    # bias, into one vector added after the row-parallel partials are summed
    # (the Megatron bias-after-all-reduce placement).
    b_attn = np.asarray(b_attn, np.float64)
    return (
        np.asarray(b_proj, np.float64)
        + b_attn[2 * C : 3 * C] @ np.asarray(w_proj, np.float64)
    ).astype(np.float64)


def kernel(x, w_attn, b_attn, w_proj, b_proj, last_k_no_attend=0, window_size=0):
    from concourse.bass_utils import run_bass_kernel_spmd

    nc = _get_nc()
    maps = _prep_in_maps(x, w_attn, b_attn, w_proj, b_proj)
    res = run_bass_kernel_spmd(nc, maps, list(range(N_CORES)))
    return _gather(res.results, _bias_eff(b_attn, w_proj, b_proj))

